# revision 6
# baseline (speedup 1.0000x reference)
"""Causal self-attention Trainium2 kernel (B=8, T=1024, C=768, H=12 heads).

Strategy: data-parallel over batch — one batch element per NeuronCore (8 cores).
Per core, everything is computed in a "transposed" layout so that no on-device
transposes are needed:

  qT, kT  [C, T]   = w_attn_{q,k}.T @ x.T          (x.T supplied by host)
  v_aug   [T, 780] = x @ [w_attn_v | 0]  (+ ones column per head, stride 65)
  sT_h    [Tk, Tq] = kT_h.T-slices @ qT_h          (keys on partitions, the two
                                                    heads of a pair run as
                                                    concurrent row-tiled MMs)
  eT      = exp(sT / 8), bf16, causal mask via one batched 2-head multiply
  yT_aug  [65, Tq] = v_aug_h.T @ eT                (row 64 = softmax row-sums)
  yT_norm = yT * broadcast(1/sums)                 (broadcast via K=2 matmul)
  out     [T, C]   = yT_norm.T-slices @ w_proj

Matmuls run fp32r (reduced-precision fp32 PE mode) except attv (bf16 exp/v).

The issue order forms a software pipeline: the scalar-engine exp latency
((N+352)/1.2 ns) is hidden by placing QK / v / projection matmuls between the
score and attv matmuls of each head pair.  Phase alpha covers query half 0
plus all QK tiles; phase beta covers query half 1 plus the tail v tiles and
the output projection (contraction split 0-2 / 3-5 so the final dependency
chain after the last attention block stays short).  Per-head-pair
normalization is deferred by one block so the PE queue never waits on the
sums DMA chain.
"""
import sys

sys.path.insert(0, "/opt/trn_rl_repo")

import ml_dtypes
import numpy as np

import concourse.bass as bass
import concourse.bacc as bacc
import concourse.tile as tile
import concourse.mybir as mybir
from concourse.bass_utils import run_bass_kernel_spmd

f32 = mybir.dt.float32
f32r = mybir.dt.float32r
bf16 = mybir.dt.bfloat16
EXP = mybir.ActivationFunctionType.Exp

B, T, C = 8, 1024, 768
H, D = 12, 64
DA = D + 1        # per-head block in v: [v_h(64) | 1]
HB = 2 * DA       # head-pair stride
VW = H * DA       # 780
NK = C // 128     # 6 contraction tiles
NT = T // 128     # 8 token tiles
SCALE = 1.0 / np.sqrt(D)


def build():
    nc = bacc.Bacc("TRN2", target_bir_lowering=False, debug=False)
    xT = nc.dram_tensor("xT", [C, T], f32r, kind="ExternalInput")
    wq = nc.dram_tensor("wq", [2 * NK, 128, NK, 128], f32r, kind="ExternalInput")
    wv = nc.dram_tensor("wv", [NK, 128, VW], f32r, kind="ExternalInput")
    wp = nc.dram_tensor("wp", [NK, 128, C], f32r, kind="ExternalInput")
    msk = nc.dram_tensor("msk", [128, 256], bf16, kind="ExternalInput")
    onesc = nc.dram_tensor("onesc", [128, H], bf16, kind="ExternalInput")
    sel = nc.dram_tensor("sel", [2, C], f32r, kind="ExternalInput")
    out = nc.dram_tensor("out", [T, C], f32, kind="ExternalOutput")

    with tile.TileContext(nc) as tc:
        with (
            tc.tile_pool(name="const", bufs=1) as const,
            tc.tile_pool(name="wqp", bufs=2) as wqp,
            tc.tile_pool(name="exp", bufs=4) as expp,
            tc.tile_pool(name="spp", bufs=2, space="PSUM") as spp,
            tc.tile_pool(name="fil", bufs=1, space="PSUM") as fil,
            tc.tile_pool(name="psm", bufs=2, space="PSUM") as psm,
        ):
            # ---- resident SBUF tensors ----
            xTall = const.tile([128, NK * T], f32r, tag="xTall")
            xT_t = [xTall[:, i * T:(i + 1) * T] for i in range(NK)]
            wvall = const.tile([128, NK * VW], f32r, tag="wvall")
            wv_t = [wvall[:, i * VW:(i + 1) * VW] for i in range(NK)]
            wpall = const.tile([128, NK * C], f32r, tag="wpall")
            wp_t = [wpall[:, i * C:(i + 1) * C] for i in range(NK)]
            qkT_t = [const.tile([128, T], f32r, name=f"qks{m}", tag=f"qk{m}") for m in range(2 * NK)]
            v_t = [const.tile([128, VW], bf16, name=f"vs{t}", tag=f"v{t}") for t in range(NT)]
            yT_t = [const.tile([128, T], f32r, name=f"yTs{i}", tag=f"yT{i}") for i in range(NK)]
            part1 = [const.tile([128, C], f32, name=f"p1s{t}", tag=f"p1{t}") for t in range(4)]
            msk_t = const.tile([128, 256], bf16, tag="msk")
            ones_t = const.tile([128, H], bf16, tag="ones")
            sel_t = const.tile([2, C], f32r, tag="sel")
            # softmax sums for head pair hp at [2, hp*512:(hp+1)*512]; the
            # region is reused across the two query halves (DVE reciprocal
            # needs start partition 0)
            sums_t = const.tile([2, NK * 512], f32, tag="sums")
            rec_t = const.tile([2, 512], f32r, tag="rec")

            xTd = xT.rearrange("(i p) n -> p i n", p=128)

            # ---------------- building blocks ----------------
            qk_ps = {}

            def qk_start(m, wq_t, kks):
                """Accumulation MMs for qk tile m over contraction tiles kks
                (kk-outer, both query halves inner => x tiles consumed in DMA
                arrival order)."""
                if m not in qk_ps:
                    qk_ps[m] = fil.tile([128, 1024], f32, tag="fil", name=f"psqk{m}")
                ps = qk_ps[m]
                for kk in kks:
                    for qc in range(2):
                        nc.tensor.matmul(
                            ps[:, qc * 512:(qc + 1) * 512],
                            wq_t[:, kk, :],
                            xT_t[kk][:, qc * 512:(qc + 1) * 512],
                            start=(kk == 0),
                            stop=(kk == NK - 1),
                        )

            def qk_finish(m):
                ps = qk_ps.pop(m)
                nc.scalar.copy(qkT_t[m], ps)

            def v_tile(t):
                ps = fil.tile([128, 1024], f32, tag="fil", name=f"psv{t}")
                for n0, nw in ((0, 512), (512, VW - 512)):
                    for kk in range(NK):
                        nc.tensor.matmul(
                            ps[:, n0:n0 + nw],
                            xT_t[kk][:, t * 128:(t + 1) * 128],
                            wv_t[kk][:, n0:n0 + nw],
                            start=(kk == 0),
                            stop=(kk == NK - 1),
                        )
                nc.vector.tensor_copy(v_t[t], ps[:, :VW])
                ones_ap = v_t[t].rearrange("p (h e) -> p h e", e=DA)[:, :, D]
                nc.vector.tensor_copy(ones_ap, ones_t)

            yps = {}
            exs = {}

            def S(hp, qc, kt):
                """Scores pair (row-tiled, concurrent) + exp (+ causal mask)."""
                qT = qkT_t[hp]
                kT = qkT_t[NK + hp]
                ks = slice(kt * 128, (kt + 1) * 128)
                pos = max(kt * 128 - qc * 512, 0)
                qv = slice(qc * 512 + pos, (qc + 1) * 512)
                sp = spp.tile([128, 1024], f32, tag="sp", name="sp")
                nc.tensor.matmul(
                    sp[:, pos:512], kT[0:64, ks], qT[0:64, qv],
                    start=True, stop=True,
                )
                nc.tensor.matmul(
                    sp[:, 512 + pos:1024], kT[64:128, ks], qT[64:128, qv],
                    start=True, stop=True,
                )
                ex = expp.tile([128, 1024], bf16, tag="ex", name="ex")
                if pos == 0:
                    nc.scalar.activation(ex, sp, EXP, scale=float(SCALE))
                else:
                    exv = ex.rearrange("p (i n) -> p i n", i=2)[:, :, pos:512]
                    spv = sp.rearrange("p (i n) -> p i n", i=2)[:, :, pos:512]
                    nc.scalar.activation(exv, spv, EXP, scale=float(SCALE))
                if kt * 128 >= qc * 512:  # diagonal tile: mask both heads at once
                    exd = ex.rearrange("p (i n) -> p i n", i=2)[:, :, pos:pos + 128]
                    mkd = msk_t.rearrange("p (i n) -> p i n", i=2)
                    nc.vector.tensor_mul(exd, exd, mkd)
                exs[(hp, qc, kt)] = (ex, pos)

            def A(hp, qc, kt, nkt):
                """attv pair for exp tile (hp, qc, kt)."""
                if (hp, qc) not in yps:
                    yps[(hp, qc)] = (
                        psm.tile([128, 512], f32, tag="yp", name="ypA"),
                        psm.tile([128, 512], f32, tag="yp", name="ypB"),
                    )
                ypA, ypB = yps[(hp, qc)]
                ex, pos = exs.pop((hp, qc, kt))
                for yp, half in ((ypA, 0), (ypB, 1)):
                    nc.tensor.matmul(
                        yp[:DA, pos:512],
                        v_t[kt][:, hp * HB + half * DA:hp * HB + (half + 1) * DA],
                        ex[:, half * 512 + pos:(half + 1) * 512],
                        start=(kt == 0), stop=(kt == nkt - 1),
                    )

            def FIN_stage(hp, qc):
                """Stage attv outputs to SBUF, DMA y rows + softmax sums out."""
                qs = slice(qc * 512, (qc + 1) * 512)
                hs = slice(hp * 512, (hp + 1) * 512)
                ypA, ypB = yps.pop((hp, qc))
                for r, (yp, off) in enumerate(((ypA, 0), (ypB, 64))):
                    stage = expp.tile([DA, 512], f32r, tag="ystage", bufs=2, name="stage")
                    nc.vector.tensor_copy(stage, yp[:DA, :])
                    nc.sync.dma_start(out=yT_t[hp][off:off + 64, qs], in_=stage[:D, :])
                    nc.sync.dma_start(
                        out=sums_t[r:r + 1, hs], in_=stage[D:DA, :].bitcast(f32)
                    )

            def FIN_norm(hp, qc):
                """Normalize yT rows of this head pair (deferred >= 1 block so
                the PE queue never waits on the sums DMA chain)."""
                qs = slice(qc * 512, (qc + 1) * 512)
                hs = slice(hp * 512, (hp + 1) * 512)
                nc.vector.reciprocal_approx_fast(sums_t[:, hs], sums_t[:, hs])
                with nc.allow_low_precision(reason="f32r recip feeds f32r matmul"):
                    nc.vector.tensor_copy(rec_t, sums_t[:, hs])
                bc = fil.tile([128, 512], f32, tag="fil", name="bc")
                nc.tensor.matmul(bc, sel_t[:, hp * 128:(hp + 1) * 128], rec_t,
                                 start=True, stop=True)
                nc.vector.tensor_mul(yT_t[hp][:, qs], yT_t[hp][:, qs], bc.bitcast(f32r))

            def P_full(t):
                pp = fil.tile([128, 1024], f32, tag="fil", name=f"pp{t}")
                for n0, nw in ((0, 512), (512, 256)):
                    for kk in range(NK):
                        nc.tensor.matmul(
                            pp[:, n0:n0 + nw],
                            yT_t[kk][:, t * 128:(t + 1) * 128],
                            wp_t[kk][:, n0:n0 + nw],
                            start=(kk == 0),
                            stop=(kk == NK - 1),
                        )
                ostage = expp.tile([128, C], f32, tag="ostage", bufs=2, name="ostage")
                nc.vector.tensor_copy(ostage, pp[:, :C])
                nc.sync.dma_start(out=out[t * 128:(t + 1) * 128, :], in_=ostage)

            def P1(t):
                """Early half of projection tile t: contraction tiles 0..2."""
                pp = fil.tile([128, 1024], f32, tag="fil", name=f"pp1{t}")
                for n0, nw in ((0, 512), (512, 256)):
                    for kk in range(3):
                        nc.tensor.matmul(
                            pp[:, n0:n0 + nw],
                            yT_t[kk][:, t * 128:(t + 1) * 128],
                            wp_t[kk][:, n0:n0 + nw],
                            start=(kk == 0),
                            stop=(kk == 2),
                        )
                nc.vector.tensor_copy(part1[t - 4], pp[:, :C])

            def P2(t):
                """Late half of projection tile t: contraction tiles 3..5 + merge."""
                pp = fil.tile([128, 1024], f32, tag="fil", name=f"pp2{t}")
                for n0, nw in ((0, 512), (512, 256)):
                    for kk in range(3, NK):
                        nc.tensor.matmul(
                            pp[:, n0:n0 + nw],
                            yT_t[kk][:, t * 128:(t + 1) * 128],
                            wp_t[kk][:, n0:n0 + nw],
                            start=(kk == 3),
                            stop=(kk == NK - 1),
                        )
                ostage = expp.tile([128, C], f32, tag="ostage", bufs=2, name="ostage")
                nc.vector.tensor_add(ostage, pp[:, :C], part1[t - 4])
                nc.sync.dma_start(out=out[t * 128:(t + 1) * 128, :], in_=ostage)

            # ---------------- schedule ----------------
            wq_tiles = {}

            def wq_fetch(m):
                wq_tiles[m] = wqp.tile([128, NK, 128], f32r, tag="wq", name=f"wq{m}")
                nc.sync.dma_start(out=wq_tiles[m], in_=wq[m, :, :, :])

            # Head: interleave x contraction-tile DMAs (half1+half2 per kk) so
            # the first QK matmuls (kk-outer) start as soon as kk=0 lands.
            wq_fetch(0)
            for kk in range(NK):
                nc.sync.dma_start(out=xT_t[kk][:, 0:512], in_=xTd[:, kk, 0:512])
                nc.sync.dma_start(out=xT_t[kk][:, 512:1024], in_=xTd[:, kk, 512:1024])
            wq_fetch(6)

            qk_start(0, wq_tiles[0], range(NK))
            qk_finish(0)
            nc.sync.dma_start(
                out=wvall.rearrange("p (i n) -> p i n", i=NK),
                in_=wv.rearrange("i p n -> p i n"),
            )
            nc.sync.dma_start(out=msk_t, in_=msk[:, :])
            nc.sync.dma_start(out=ones_t, in_=onesc[:, :])
            nc.sync.dma_start(out=sel_t, in_=sel[:, :])
            qk_start(6, wq_tiles.pop(6), range(NK))
            qk_finish(6)
            wq_tiles.pop(0)
            for t in range(4):
                if t == 2:
                    wq_fetch(1)
                v_tile(t)

            # ---- alpha: query half 0 attention + remaining QK tiles ----
            for hp in range(NK):
                S(hp, 0, 0)
                S(hp, 0, 1)
                if hp > 0:
                    FIN_norm(hp - 1, 0)
                if hp < 5:
                    m = hp + 1
                    qk_start(m, wq_tiles[m], range(3))
                    wq_fetch(NK + hp + 1)
                    qk_start(m, wq_tiles.pop(m), range(3, NK))
                    qk_finish(m)
                else:
                    v_tile(4)
                A(hp, 0, 0, 4)
                S(hp, 0, 2)
                A(hp, 0, 1, 4)
                S(hp, 0, 3)
                if hp < 5:
                    m = NK + hp + 1
                    qk_start(m, wq_tiles[m], range(3))
                    if hp == 1:
                        nc.sync.dma_start(
                            out=wpall.rearrange("p (i n) -> p i n", i=NK),
                            in_=wp.rearrange("i p n -> p i n"),
                        )
                    if hp < 4:
                        wq_fetch(hp + 2)
                    qk_start(m, wq_tiles.pop(m), range(3, NK))
                    qk_finish(m)
                A(hp, 0, 2, 4)
                A(hp, 0, 3, 4)
                FIN_stage(hp, 0)

            # ---- beta: query half 1 attention + v tail + projection ----
            beta_fill = {
                0: [lambda: v_tile(5), lambda: v_tile(6), lambda: v_tile(7)],
                1: [lambda: P_full(0), lambda: P_full(1)],
                2: [lambda: P_full(2), lambda: P_full(3)],
                3: [lambda: P1(4), lambda: P1(5)],
                4: [lambda: P1(6), lambda: P1(7)],
                5: [],
            }
            for hp in range(NK):
                fills = list(beta_fill[hp])

                def fill():
                    if fills:
                        fills.pop(0)()

                S(hp, 1, 0)
                S(hp, 1, 1)
                if hp == 0:
                    FIN_norm(5, 0)
                else:
                    FIN_norm(hp - 1, 1)
                fill()
                A(hp, 1, 0, 8)
                S(hp, 1, 2)
                A(hp, 1, 1, 8)
                S(hp, 1, 3)
                fill()
                A(hp, 1, 2, 8)
                S(hp, 1, 4)
                A(hp, 1, 3, 8)
                S(hp, 1, 5)
                fill()
                A(hp, 1, 4, 8)
                S(hp, 1, 6)
                A(hp, 1, 5, 8)
                S(hp, 1, 7)
                A(hp, 1, 6, 8)
                A(hp, 1, 7, 8)
                FIN_stage(hp, 1)

            FIN_norm(5, 1)
            for t in range(4, NT):
                P2(t)

    nc.compile()
    return nc


_nc = None


def _get_nc():
    global _nc
    if _nc is None:
        _nc = build()
    return _nc


def _host_prep(w_attn, w_proj):
    wq = np.ascontiguousarray(
        w_attn[:, :2 * C].reshape(NK, 128, 2 * NK, 128).transpose(2, 1, 0, 3)
    )
    wv_aug = np.zeros((C, H, DA), np.float32)
    wv_aug[:, :, :D] = w_attn[:, 2 * C:].reshape(C, H, D)
    wv = np.ascontiguousarray(wv_aug.reshape(NK, 128, VW))
    wp = np.ascontiguousarray(w_proj.reshape(NK, 128, C))
    tri = np.triu(np.ones((128, 128), np.float32))
    msk = np.concatenate([tri, tri], axis=1).astype(ml_dtypes.bfloat16)
    onesc = np.ones((128, H), ml_dtypes.bfloat16)
    sel = np.zeros((2, C), np.float32)
    for p in range(C):
        sel[(p % 128) // 64, p] = 1.0
    return wq, wv, wp, msk, onesc, sel


def kernel(x, w_attn, w_proj):
    x = np.asarray(x, dtype=np.float32)
    w_attn = np.asarray(w_attn, dtype=np.float32)
    w_proj = np.asarray(w_proj, dtype=np.float32)
    wq, wv, wp, msk, onesc, sel = _host_prep(w_attn, w_proj)
    in_maps = [
        {
            "xT": np.ascontiguousarray(x[b].T),
            "wq": wq,
            "wv": wv,
            "wp": wp,
            "msk": msk,
            "onesc": onesc,
            "sel": sel,
        }
        for b in range(B)
    ]
    last_err = None
    for _attempt in range(3):
        try:
            res = run_bass_kernel_spmd(_get_nc(), in_maps, list(range(B)))
            return np.stack([res.results[b]["out"] for b in range(B)], axis=0)
        except Exception as e:  # transient device wedge: retry
            last_err = e
    raise last_err


# revision 11
# speedup vs baseline: 1.0418x; 1.0418x over previous
"""Causal self-attention Trainium2 kernel (B=8, T=1024, C=768, H=12 heads).

Strategy: data-parallel over batch — one batch element per NeuronCore (8 cores).
Per core, everything is computed in a "transposed" layout so that no on-device
transposes are needed:

  qT, kT  [C, T]   = w_attn_{q,k}.T @ x.T          (x.T supplied by host)
  v_aug   [T, 780] = x @ [w_attn_v | 0]  (+ ones column per head, stride 65)
  sT_h    [Tk, Tq] = kT_h.T-slices @ qT_h          (keys on partitions, the two
                                                    heads of a pair run as
                                                    concurrent row-tiled MMs)
  eT      = exp(sT / 8), bf16, causal mask via one batched 2-head multiply
  yT_aug  [65, Tq] = v_aug_h.T @ eT                (row 64 = softmax row-sums)
  yT_norm = yT * broadcast(1/sums)                 (broadcast via K=2 matmul)
  out     [T, C]   = yT_norm.T-slices @ w_proj

Matmuls run fp32r (reduced-precision fp32 PE mode) except attv (bf16 exp/v).

The issue order forms a software pipeline tuned so no engine starves:
 - phase alpha: all 12 QK tiles + all 8 v tiles + query-half-0 attention,
   with QK/v matmuls interleaved between score and attv matmuls to hide the
   scalar-engine exp latency ((N+352)/1.2 ns per tile);
 - phase beta: query-half-1 attention with the output projection as filler.
   The projection is split by contraction (heads 0-2 -> bf16 SBUF partial,
   heads 3-4 as late filler, head 5 in the tail) so the dependency chain
   after the last attention block stays short.
Per-head-pair normalization is deferred by one block so the in-order PE queue
never waits on the sums DMA chain.
"""
import sys

sys.path.insert(0, "/opt/trn_rl_repo")

import ml_dtypes
import numpy as np

import concourse.bass as bass
import concourse.bacc as bacc
import concourse.tile as tile
import concourse.mybir as mybir
from concourse.bass_utils import run_bass_kernel_spmd

f32 = mybir.dt.float32
f32r = mybir.dt.float32r
bf16 = mybir.dt.bfloat16
EXP = mybir.ActivationFunctionType.Exp

B, T, C = 8, 1024, 768
H, D = 12, 64
DA = D + 1        # per-head block in v: [v_h(64) | 1]
HB = 2 * DA       # head-pair stride
VW = H * DA       # 780
NK = C // 128     # 6 contraction tiles
NT = T // 128     # 8 token tiles
SCALE = 1.0 / np.sqrt(D)


def build():
    nc = bacc.Bacc("TRN2", target_bir_lowering=False, debug=False)
    xT = nc.dram_tensor("xT", [C, T], f32r, kind="ExternalInput")
    wq = nc.dram_tensor("wq", [2 * NK, 128, NK, 128], f32r, kind="ExternalInput")
    wv = nc.dram_tensor("wv", [NK, 128, VW], f32r, kind="ExternalInput")
    wp = nc.dram_tensor("wp", [NK, 128, C], f32r, kind="ExternalInput")
    msk = nc.dram_tensor("msk", [128, 256], bf16, kind="ExternalInput")
    onesc = nc.dram_tensor("onesc", [128, H], bf16, kind="ExternalInput")
    sel = nc.dram_tensor("sel", [2, C], f32r, kind="ExternalInput")
    out = nc.dram_tensor("out", [T, C], f32, kind="ExternalOutput")

    with tile.TileContext(nc) as tc:
        with (
            tc.tile_pool(name="const", bufs=1) as const,
            tc.tile_pool(name="wqp", bufs=3) as wqp,
            tc.tile_pool(name="exp", bufs=4) as expp,
            tc.tile_pool(name="spp", bufs=2, space="PSUM") as spp,
            tc.tile_pool(name="fil", bufs=1, space="PSUM") as fil,
            tc.tile_pool(name="psm", bufs=2, space="PSUM") as psm,
        ):
            # ---- resident SBUF tensors ----
            xTall = const.tile([128, NK * T], f32r, tag="xTall")
            xT_t = [xTall[:, i * T:(i + 1) * T] for i in range(NK)]
            wvall = const.tile([128, NK * VW], f32r, tag="wvall")
            wv_t = [wvall[:, i * VW:(i + 1) * VW] for i in range(NK)]
            wvd = wvall.rearrange("p (i n) -> p i n", i=NK)
            wpall = const.tile([128, NK * C], f32r, tag="wpall")
            wp_t = [wpall[:, i * C:(i + 1) * C] for i in range(NK)]
            qkT_t = [const.tile([128, T], f32r, name=f"qks{m}", tag=f"qk{m}") for m in range(2 * NK)]
            v_t = [const.tile([128, VW], bf16, name=f"vs{t}", tag=f"v{t}") for t in range(NT)]
            yT_t = [const.tile([128, T], f32r, name=f"yTs{i}", tag=f"yT{i}") for i in range(NK)]
            part = [const.tile([128, C], bf16, name=f"prt{t}", tag=f"prt{t}") for t in range(NT)]
            msk_t = const.tile([128, 256], bf16, tag="msk")
            ones_t = const.tile([128, H], bf16, tag="ones")
            sel_t = const.tile([2, C], f32r, tag="sel")
            # softmax sums for head pair hp at [2, hp*512:(hp+1)*512]; region
            # reused across the two query halves (DVE recip needs partition 0)
            sums_t = const.tile([2, NK * 512], f32, tag="sums")
            rec_t = const.tile([2, 512], f32r, tag="rec")

            xTd = xT.rearrange("(i p) n -> p i n", p=128)
            wvs = wv.rearrange("i p n -> p i n")

            # ---------------- building blocks ----------------
            qk_ps = {}

            def qk_start(m, wq_t, kks):
                """Accumulation MMs for qk tile m over contraction tiles kks
                (kk-outer so x tiles are consumed in DMA arrival order)."""
                if m not in qk_ps:
                    qk_ps[m] = fil.tile([128, 1024], f32, tag="fil", name=f"psqk{m}")
                ps = qk_ps[m]
                for kk in kks:
                    for qc in range(2):
                        nc.tensor.matmul(
                            ps[:, qc * 512:(qc + 1) * 512],
                            wq_t[:, kk, :],
                            xT_t[kk][:, qc * 512:(qc + 1) * 512],
                            start=(kk == 0),
                            stop=(kk == NK - 1),
                        )

            def qk_finish(m):
                ps = qk_ps.pop(m)
                if m % 2 == 0:
                    nc.scalar.copy(qkT_t[m], ps)
                else:
                    nc.vector.tensor_copy(qkT_t[m], ps)

            def v_tile(t):
                ps = fil.tile([128, 1024], f32, tag="fil", name=f"psv{t}")
                for n0, nw in ((0, 512), (512, VW - 512)):
                    for kk in range(NK):
                        nc.tensor.matmul(
                            ps[:, n0:n0 + nw],
                            xT_t[kk][:, t * 128:(t + 1) * 128],
                            wv_t[kk][:, n0:n0 + nw],
                            start=(kk == 0),
                            stop=(kk == NK - 1),
                        )
                nc.vector.tensor_copy(v_t[t], ps[:, :VW])
                ones_ap = v_t[t].rearrange("p (h e) -> p h e", e=DA)[:, :, D]
                nc.vector.tensor_copy(ones_ap, ones_t)

            yps = {}
            exs = {}

            def S(hp, qc, kt):
                """Scores pair (row-tiled, concurrent) + exp (+ causal mask)."""
                qT = qkT_t[hp]
                kT = qkT_t[NK + hp]
                ks = slice(kt * 128, (kt + 1) * 128)
                pos = max(kt * 128 - qc * 512, 0)
                qv = slice(qc * 512 + pos, (qc + 1) * 512)
                sp = spp.tile([128, 1024], f32, tag="sp", name="sp")
                nc.tensor.matmul(
                    sp[:, pos:512], kT[0:64, ks], qT[0:64, qv],
                    start=True, stop=True,
                )
                nc.tensor.matmul(
                    sp[:, 512 + pos:1024], kT[64:128, ks], qT[64:128, qv],
                    start=True, stop=True,
                )
                ex = expp.tile([128, 1024], bf16, tag="ex", name="ex")
                if pos == 0:
                    nc.scalar.activation(ex, sp, EXP, scale=float(SCALE))
                else:
                    exv = ex.rearrange("p (i n) -> p i n", i=2)[:, :, pos:512]
                    spv = sp.rearrange("p (i n) -> p i n", i=2)[:, :, pos:512]
                    nc.scalar.activation(exv, spv, EXP, scale=float(SCALE))
                if kt * 128 >= qc * 512:  # diagonal tile: mask both heads at once
                    exd = ex.rearrange("p (i n) -> p i n", i=2)[:, :, pos:pos + 128]
                    mkd = msk_t.rearrange("p (i n) -> p i n", i=2)
                    nc.vector.tensor_mul(exd, exd, mkd)
                exs[(hp, qc, kt)] = (ex, pos)

            def A(hp, qc, kt, nkt):
                """attv pair for exp tile (hp, qc, kt)."""
                if (hp, qc) not in yps:
                    yps[(hp, qc)] = (
                        psm.tile([128, 512], f32, tag="yp", name="ypA"),
                        psm.tile([128, 512], f32, tag="yp", name="ypB"),
                    )
                ypA, ypB = yps[(hp, qc)]
                ex, pos = exs.pop((hp, qc, kt))
                for yp, half in ((ypA, 0), (ypB, 1)):
                    nc.tensor.matmul(
                        yp[:DA, pos:512],
                        v_t[kt][:, hp * HB + half * DA:hp * HB + (half + 1) * DA],
                        ex[:, half * 512 + pos:(half + 1) * 512],
                        start=(kt == 0), stop=(kt == nkt - 1),
                    )

            def FIN_stage(hp, qc, sums_first=False):
                """Stage attv outputs to SBUF, DMA y rows + softmax sums out."""
                qs = slice(qc * 512, (qc + 1) * 512)
                hs = slice(hp * 512, (hp + 1) * 512)
                for r, (yp, off) in enumerate(((yps[(hp, qc)][0], 0), (yps[(hp, qc)][1], 64))):
                    stage = expp.tile([DA, 512], f32r, tag="ystage", bufs=2, name="stage")
                    if sums_first:
                        nc.vector.tensor_copy(stage[D:DA, :], yp[D:DA, :])
                        nc.sync.dma_start(
                            out=sums_t[r:r + 1, hs], in_=stage[D:DA, :].bitcast(f32)
                        )
                        nc.vector.tensor_copy(stage[:D, :], yp[:D, :])
                        nc.sync.dma_start(out=yT_t[hp][off:off + 64, qs], in_=stage[:D, :])
                    else:
                        nc.vector.tensor_copy(stage, yp[:DA, :])
                        nc.sync.dma_start(out=yT_t[hp][off:off + 64, qs], in_=stage[:D, :])
                        nc.sync.dma_start(
                            out=sums_t[r:r + 1, hs], in_=stage[D:DA, :].bitcast(f32)
                        )
                del yps[(hp, qc)]

            def FIN_norm(hp, qc):
                """Normalize yT rows of this head pair (deferred >= 1 block so
                the PE queue never waits on the sums DMA chain)."""
                qs = slice(qc * 512, (qc + 1) * 512)
                hs = slice(hp * 512, (hp + 1) * 512)
                nc.vector.reciprocal_approx_fast(sums_t[:, hs], sums_t[:, hs])
                with nc.allow_low_precision(reason="f32r recip feeds f32r matmul"):
                    nc.vector.tensor_copy(rec_t, sums_t[:, hs])
                bc = fil.tile([128, 512], f32, tag="fil", name="bc")
                nc.tensor.matmul(
                    bc, sel_t[:, hp * 128:(hp + 1) * 128], rec_t,
                    start=True, stop=True,
                )
                nc.vector.tensor_mul(yT_t[hp][:, qs], yT_t[hp][:, qs], bc.bitcast(f32r))

            def P_mm(pool, t, kk0, kk1):
                pp = pool.tile([128, 1024], f32, tag="fil" if pool is fil else "sp",
                               name=f"pp{t}_{kk0}")
                for n0, nw in ((0, 512), (512, 256)):
                    for kk in range(kk0, kk1):
                        nc.tensor.matmul(
                            pp[:, n0:n0 + nw],
                            yT_t[kk][:, t * 128:(t + 1) * 128],
                            wp_t[kk][:, n0:n0 + nw],
                            start=(kk == kk0),
                            stop=(kk == kk1 - 1),
                        )
                return pp

            def Pa(t):
                """Projection tile t, contraction 0..2 -> bf16 partial."""
                pp = P_mm(fil, t, 0, 3)
                nc.vector.tensor_copy(part[t], pp[:, :C])

            def P2a(t):
                """Projection tile t, contraction 3..4 added into the partial."""
                pp = P_mm(fil, t, 3, 5)
                nc.vector.tensor_add(part[t], pp[:, :C], part[t])

            def Pb(t, kk0, pool):
                """Projection tile t, contraction kk0..5 + merge partial + out."""
                pp = P_mm(pool, t, kk0, NK)
                ostage = expp.tile([128, C], f32, tag="ostage", bufs=2, name="ostage")
                nc.vector.tensor_add(ostage, pp[:, :C], part[t])
                nc.sync.dma_start(out=out[t * 128:(t + 1) * 128, :], in_=ostage)

            # ---------------- schedule ----------------
            wq_tiles = {}

            def wq_fetch(m):
                wq_tiles[m] = wqp.tile([128, NK, 128], f32r, tag="wq", name=f"wq{m}")
                nc.sync.dma_start(out=wq_tiles[m], in_=wq[m, :, :, :])

            # Head: wq0/wq6 first (small, unblock the first two QK tiles), then
            # x in per-kk chunks with the constants and wv thirds woven between.
            wq_fetch(0)
            wq_fetch(6)
            for kk in range(NK):
                nc.sync.dma_start(out=xT_t[kk][:, 0:512], in_=xTd[:, kk, 0:512])
                nc.sync.dma_start(out=xT_t[kk][:, 512:1024], in_=xTd[:, kk, 512:1024])
                if kk == 1:
                    nc.sync.dma_start(out=msk_t, in_=msk[:, :])
                    nc.sync.dma_start(out=ones_t, in_=onesc[:, :])
                    nc.sync.dma_start(out=sel_t, in_=sel[:, :])
                elif kk >= 3:
                    i = kk - 3
                    nc.sync.dma_start(out=wvd[:, 2 * i:2 * i + 2, :], in_=wvs[:, 2 * i:2 * i + 2, :])

            qk_start(0, wq_tiles[0], range(NK))
            qk_finish(0)
            wq_fetch(1)
            qk_start(6, wq_tiles.pop(6), range(NK))
            qk_finish(6)
            wq_tiles.pop(0)
            wq_fetch(7)
            for t in range(4):
                v_tile(t)
                if t == 1:
                    wq_fetch(2)

            # ---- alpha: query half 0 attention + QK tiles + v tiles ----
            for hp in range(NK):
                S(hp, 0, 0)
                S(hp, 0, 1)
                if hp > 0:
                    FIN_norm(hp - 1, 0)
                if hp < 5:
                    m = hp + 1
                    qk_start(m, wq_tiles[m], range(3))
                    if hp < 4:
                        wq_fetch(NK + hp + 2)
                    qk_start(m, wq_tiles.pop(m), range(3, NK))
                    qk_finish(m)
                else:
                    v_tile(7)
                A(hp, 0, 0, 4)
                S(hp, 0, 2)
                A(hp, 0, 1, 4)
                S(hp, 0, 3)
                if hp < 5:
                    m = NK + hp + 1
                    qk_start(m, wq_tiles[m], range(3))
                    if hp == 1:
                        nc.sync.dma_start(
                            out=wpall.rearrange("p (i n) -> p i n", i=NK),
                            in_=wp.rearrange("i p n -> p i n"),
                        )
                    if hp < 3:
                        wq_fetch(hp + 3)
                    qk_start(m, wq_tiles.pop(m), range(3, NK))
                    qk_finish(m)
                A(hp, 0, 2, 4)
                A(hp, 0, 3, 4)
                if 2 <= hp <= 4:
                    v_tile(hp + 2)
                FIN_stage(hp, 0)

            # ---- beta: query half 1 attention + projection filler ----
            beta_fill = {
                0: [lambda: Pa(0), lambda: Pa(1), lambda: Pb(0, 3, fil)],
                1: [lambda: Pa(2), lambda: Pa(3), lambda: Pb(1, 3, fil)],
                2: [lambda: Pb(2, 3, fil), lambda: Pb(3, 3, fil)],
                3: [lambda: Pa(4), lambda: Pa(5)],
                4: [lambda: Pa(6), lambda: Pa(7)],
                5: [lambda: P2a(4), lambda: P2a(5)],
            }
            for hp in range(NK):
                fills = list(beta_fill[hp])

                def fill():
                    if fills:
                        fills.pop(0)()

                S(hp, 1, 0)
                S(hp, 1, 1)
                if hp == 0:
                    FIN_norm(5, 0)
                else:
                    FIN_norm(hp - 1, 1)
                fill()
                A(hp, 1, 0, 8)
                S(hp, 1, 2)
                A(hp, 1, 1, 8)
                S(hp, 1, 3)
                fill()
                A(hp, 1, 2, 8)
                S(hp, 1, 4)
                A(hp, 1, 3, 8)
                S(hp, 1, 5)
                fill()
                A(hp, 1, 4, 8)
                S(hp, 1, 6)
                A(hp, 1, 5, 8)
                S(hp, 1, 7)
                A(hp, 1, 6, 8)
                A(hp, 1, 7, 8)
                FIN_stage(hp, 1, sums_first=(hp == 5))

            FIN_norm(5, 1)
            Pb(4, 5, spp)
            Pb(5, 5, spp)
            Pb(6, 3, spp)
            Pb(7, 3, spp)

    nc.compile()
    return nc


_nc = None


def _get_nc():
    global _nc
    if _nc is None:
        _nc = build()
    return _nc


def _host_prep(w_attn, w_proj):
    wq = np.ascontiguousarray(
        w_attn[:, :2 * C].reshape(NK, 128, 2 * NK, 128).transpose(2, 1, 0, 3)
    )
    wv_aug = np.zeros((C, H, DA), np.float32)
    wv_aug[:, :, :D] = w_attn[:, 2 * C:].reshape(C, H, D)
    wv = np.ascontiguousarray(wv_aug.reshape(NK, 128, VW))
    wp = np.ascontiguousarray(w_proj.reshape(NK, 128, C))
    tri = np.triu(np.ones((128, 128), np.float32))
    msk = np.concatenate([tri, tri], axis=1).astype(ml_dtypes.bfloat16)
    onesc = np.ones((128, H), ml_dtypes.bfloat16)
    sel = np.zeros((2, C), np.float32)
    for p in range(C):
        sel[(p % 128) // 64, p] = 1.0
    return wq, wv, wp, msk, onesc, sel


def kernel(x, w_attn, w_proj):
    x = np.asarray(x, dtype=np.float32)
    w_attn = np.asarray(w_attn, dtype=np.float32)
    w_proj = np.asarray(w_proj, dtype=np.float32)
    wq, wv, wp, msk, onesc, sel = _host_prep(w_attn, w_proj)
    in_maps = [
        {
            "xT": np.ascontiguousarray(x[b].T),
            "wq": wq,
            "wv": wv,
            "wp": wp,
            "msk": msk,
            "onesc": onesc,
            "sel": sel,
        }
        for b in range(B)
    ]
    last_err = None
    for _attempt in range(3):
        try:
            res = run_bass_kernel_spmd(_get_nc(), in_maps, list(range(B)))
            return np.stack([res.results[b]["out"] for b in range(B)], axis=0)
        except Exception as e:  # transient device wedge: retry
            last_err = e
    raise last_err


# revision 13
# speedup vs baseline: 1.1604x; 1.1139x over previous
"""Causal self-attention Trainium2 kernel (B=8, T=1024, C=768, H=12 heads).

Strategy: data-parallel over batch — one batch element per NeuronCore (8 cores).
Per core, everything is computed in a "transposed" layout so that no on-device
transposes are needed:

  qT, kT  [C, T]   = w_attn_{q,k}.T @ x.T          (x.T supplied by host)
  v_aug   [T, 780] = x @ [w_attn_v | 0]  (+ ones column per head, stride 65)
  sT_h    [Tk, Tq] = kT_h.T-slices @ qT_h          (keys on partitions, the two
                                                    heads of a pair run as
                                                    concurrent row-tiled MMs)
  eT      = exp(sT / 8), bf16, causal mask via one batched 2-head multiply
  yT_aug  [65, Tq] = v_aug_h.T @ eT                (row 64 = softmax row-sums)
  yT_norm = yT * broadcast(1/sums)                 (broadcast via K=2 matmul)
  out     [T, C]   = yT_norm.T-slices @ w_proj

Matmuls run fp32r (reduced-precision fp32 PE mode) except attv (bf16 exp/v).

The issue order forms a software pipeline tuned so no engine starves:
 - phase alpha: all 12 QK tiles + all 8 v tiles + query-half-0 attention,
   with QK/v matmuls interleaved between score and attv matmuls to hide the
   scalar-engine exp latency ((N+352)/1.2 ns per tile);
 - phase beta: query-half-1 attention with the output projection as filler.
   The projection is split by contraction (heads 0-2 -> bf16 SBUF partial,
   heads 3-4 as late filler, head 5 in the tail) so the dependency chain
   after the last attention block stays short.
Per-head-pair normalization is deferred by one block so the in-order PE queue
never waits on the sums DMA chain.
"""
import sys

sys.path.insert(0, "/opt/trn_rl_repo")

import ml_dtypes
import numpy as np

import concourse.bass as bass
import concourse.bacc as bacc
import concourse.tile as tile
import concourse.mybir as mybir
from concourse.bass_utils import run_bass_kernel_spmd

f32 = mybir.dt.float32
f32r = mybir.dt.float32r
bf16 = mybir.dt.bfloat16
EXP = mybir.ActivationFunctionType.Exp

B, T, C = 8, 1024, 768
H, D = 12, 64
DA = D + 1        # per-head block in v: [v_h(64) | 1]
HB = 2 * DA       # head-pair stride
VW = H * DA       # 780
NK = C // 128     # 6 contraction tiles
NT = T // 128     # 8 token tiles
SCALE = 1.0 / np.sqrt(D)


def build():
    nc = bacc.Bacc("TRN2", target_bir_lowering=False, debug=False)
    xT = nc.dram_tensor("xT", [C, T], f32r, kind="ExternalInput")
    wq = nc.dram_tensor("wq", [2 * NK, 128, NK, 128], f32r, kind="ExternalInput")
    wv = nc.dram_tensor("wv", [NK, 128, VW], f32r, kind="ExternalInput")
    wp = nc.dram_tensor("wp", [NK, 128, C], f32r, kind="ExternalInput")
    msk = nc.dram_tensor("msk", [128, 256], bf16, kind="ExternalInput")
    onesc = nc.dram_tensor("onesc", [128, H], bf16, kind="ExternalInput")
    sel = nc.dram_tensor("sel", [2, C], f32r, kind="ExternalInput")
    out = nc.dram_tensor("out", [T, C], f32, kind="ExternalOutput")

    with tile.TileContext(nc) as tc:
        with (
            tc.tile_pool(name="const", bufs=1) as const,
            tc.tile_pool(name="wqp", bufs=3) as wqp,
            tc.tile_pool(name="exp", bufs=4) as expp,
            tc.tile_pool(name="spp", bufs=2, space="PSUM") as spp,
            tc.tile_pool(name="fil", bufs=1, space="PSUM") as fil,
            tc.tile_pool(name="psm", bufs=2, space="PSUM") as psm,
        ):
            # ---- resident SBUF tensors ----
            xTall = const.tile([128, NK * T], f32r, tag="xTall")
            xT_t = [xTall[:, i * T:(i + 1) * T] for i in range(NK)]
            wvall = const.tile([128, NK * VW], f32r, tag="wvall")
            wv_t = [wvall[:, i * VW:(i + 1) * VW] for i in range(NK)]
            wvd = wvall.rearrange("p (i n) -> p i n", i=NK)
            wpall = const.tile([128, NK * C], f32r, tag="wpall")
            wp_t = [wpall[:, i * C:(i + 1) * C] for i in range(NK)]
            qkT_t = [const.tile([128, T], f32r, name=f"qks{m}", tag=f"qk{m}") for m in range(2 * NK)]
            v_t = [const.tile([128, VW], bf16, name=f"vs{t}", tag=f"v{t}") for t in range(NT)]
            yT_t = [const.tile([128, T], f32r, name=f"yTs{i}", tag=f"yT{i}") for i in range(NK)]
            part = [const.tile([128, C], bf16, name=f"prt{t}", tag=f"prt{t}") for t in range(NT)]
            msk_t = const.tile([128, 256], bf16, tag="msk")
            ones_t = const.tile([128, H], bf16, tag="ones")
            sel_t = const.tile([2, C], f32r, tag="sel")
            # softmax sums for head pair hp at [2, hp*512:(hp+1)*512]; region
            # reused across the two query halves (DVE recip needs partition 0)
            sums_t = const.tile([2, NK * 512], f32, tag="sums")
            rec_t = const.tile([2, 512], f32r, tag="rec")

            xTd = xT.rearrange("(i p) n -> p i n", p=128)
            wvs = wv.rearrange("i p n -> p i n")

            # ---------------- building blocks ----------------
            qk_ps = {}

            def qk_start(m, wq_t, kks):
                """Accumulation MMs for qk tile m over contraction tiles kks
                (kk-outer so x tiles are consumed in DMA arrival order)."""
                if m not in qk_ps:
                    qk_ps[m] = fil.tile([128, 1024], f32, tag="fil", name=f"psqk{m}")
                ps = qk_ps[m]
                for kk in kks:
                    for qc in range(2):
                        nc.tensor.matmul(
                            ps[:, qc * 512:(qc + 1) * 512],
                            wq_t[:, kk, :],
                            xT_t[kk][:, qc * 512:(qc + 1) * 512],
                            start=(kk == 0),
                            stop=(kk == NK - 1),
                        )

            def qk_finish(m):
                ps = qk_ps.pop(m)
                if m % 2 == 0:
                    nc.scalar.copy(qkT_t[m], ps)
                else:
                    nc.vector.tensor_copy(qkT_t[m], ps)

            def v_tile(t):
                ps = fil.tile([128, 1024], f32, tag="fil", name=f"psv{t}")
                for n0, nw in ((0, 512), (512, VW - 512)):
                    for kk in range(NK):
                        nc.tensor.matmul(
                            ps[:, n0:n0 + nw],
                            xT_t[kk][:, t * 128:(t + 1) * 128],
                            wv_t[kk][:, n0:n0 + nw],
                            start=(kk == 0),
                            stop=(kk == NK - 1),
                        )
                nc.vector.tensor_copy(v_t[t], ps[:, :VW])
                ones_ap = v_t[t].rearrange("p (h e) -> p h e", e=DA)[:, :, D]
                nc.vector.tensor_copy(ones_ap, ones_t)

            yps = {}
            exs = {}

            def S(hp, qc, kt):
                """Scores pair (row-tiled, concurrent) + exp (+ causal mask)."""
                qT = qkT_t[hp]
                kT = qkT_t[NK + hp]
                ks = slice(kt * 128, (kt + 1) * 128)
                pos = max(kt * 128 - qc * 512, 0)
                qv = slice(qc * 512 + pos, (qc + 1) * 512)
                sp = spp.tile([128, 1024], f32, tag="sp", name="sp")
                nc.tensor.matmul(
                    sp[:, pos:512], kT[0:64, ks], qT[0:64, qv],
                    start=True, stop=True,
                )
                nc.tensor.matmul(
                    sp[:, 512 + pos:1024], kT[64:128, ks], qT[64:128, qv],
                    start=True, stop=True,
                )
                ex = expp.tile([128, 1024], bf16, tag="ex", name="ex")
                if pos == 0:
                    nc.scalar.activation(ex, sp, EXP, scale=float(SCALE))
                else:
                    exv = ex.rearrange("p (i n) -> p i n", i=2)[:, :, pos:512]
                    spv = sp.rearrange("p (i n) -> p i n", i=2)[:, :, pos:512]
                    nc.scalar.activation(exv, spv, EXP, scale=float(SCALE))
                if kt * 128 >= qc * 512:  # diagonal tile: mask both heads at once
                    exd = ex.rearrange("p (i n) -> p i n", i=2)[:, :, pos:pos + 128]
                    mkd = msk_t.rearrange("p (i n) -> p i n", i=2)
                    nc.vector.tensor_mul(exd, exd, mkd)
                exs[(hp, qc, kt)] = (ex, pos)

            def A(hp, qc, kt, nkt):
                """attv pair for exp tile (hp, qc, kt)."""
                if (hp, qc) not in yps:
                    yps[(hp, qc)] = (
                        psm.tile([128, 512], f32, tag="yp", name="ypA"),
                        psm.tile([128, 512], f32, tag="yp", name="ypB"),
                    )
                ypA, ypB = yps[(hp, qc)]
                ex, pos = exs.pop((hp, qc, kt))
                for yp, half in ((ypA, 0), (ypB, 1)):
                    nc.tensor.matmul(
                        yp[:DA, pos:512],
                        v_t[kt][:, hp * HB + half * DA:hp * HB + (half + 1) * DA],
                        ex[:, half * 512 + pos:(half + 1) * 512],
                        start=(kt == 0), stop=(kt == nkt - 1),
                    )

            def FIN_stage(hp, qc, sums_first=False):
                """Stage attv outputs to SBUF, DMA y rows + softmax sums out."""
                qs = slice(qc * 512, (qc + 1) * 512)
                hs = slice(hp * 512, (hp + 1) * 512)
                for r, (yp, off) in enumerate(((yps[(hp, qc)][0], 0), (yps[(hp, qc)][1], 64))):
                    stage = expp.tile([DA, 512], f32r, tag="ystage", bufs=2, name="stage")
                    if sums_first:
                        nc.vector.tensor_copy(stage[D:DA, :], yp[D:DA, :])
                        nc.sync.dma_start(
                            out=sums_t[r:r + 1, hs], in_=stage[D:DA, :].bitcast(f32)
                        )
                        nc.vector.tensor_copy(stage[:D, :], yp[:D, :])
                        nc.sync.dma_start(out=yT_t[hp][off:off + 64, qs], in_=stage[:D, :])
                    else:
                        nc.vector.tensor_copy(stage, yp[:DA, :])
                        nc.sync.dma_start(out=yT_t[hp][off:off + 64, qs], in_=stage[:D, :])
                        nc.sync.dma_start(
                            out=sums_t[r:r + 1, hs], in_=stage[D:DA, :].bitcast(f32)
                        )
                del yps[(hp, qc)]

            def FIN_norm(hp, qc):
                """Normalize yT rows of this head pair (deferred >= 1 block so
                the PE queue never waits on the sums DMA chain)."""
                qs = slice(qc * 512, (qc + 1) * 512)
                hs = slice(hp * 512, (hp + 1) * 512)
                nc.vector.reciprocal_approx_fast(sums_t[:, hs], sums_t[:, hs])
                with nc.allow_low_precision(reason="f32r recip feeds f32r matmul"):
                    nc.vector.tensor_copy(rec_t, sums_t[:, hs])
                bc = fil.tile([128, 512], f32, tag="fil", name="bc")
                nc.tensor.matmul(
                    bc, sel_t[:, hp * 128:(hp + 1) * 128], rec_t,
                    start=True, stop=True,
                )
                nc.vector.tensor_mul(yT_t[hp][:, qs], yT_t[hp][:, qs], bc.bitcast(f32r))

            def P_mm(pool, t, kk0, kk1):
                pp = pool.tile([128, 1024], f32, tag="fil" if pool is fil else "sp",
                               name=f"pp{t}_{kk0}")
                for n0, nw in ((0, 512), (512, 256)):
                    for kk in range(kk0, kk1):
                        nc.tensor.matmul(
                            pp[:, n0:n0 + nw],
                            yT_t[kk][:, t * 128:(t + 1) * 128],
                            wp_t[kk][:, n0:n0 + nw],
                            start=(kk == kk0),
                            stop=(kk == kk1 - 1),
                        )
                return pp

            def Pa(t):
                """Projection tile t, contraction 0..2 -> bf16 partial."""
                pp = P_mm(fil, t, 0, 3)
                nc.vector.tensor_copy(part[t], pp[:, :C])

            def P2a(t):
                """Projection tile t, contraction 3..4 added into the partial."""
                pp = P_mm(fil, t, 3, 5)
                nc.vector.tensor_add(part[t], pp[:, :C], part[t])

            def Pb(t, kk0, pool):
                """Projection tile t, contraction kk0..5 + merge partial + out."""
                pp = P_mm(pool, t, kk0, NK)
                ostage = expp.tile([128, C], f32, tag="ostage", bufs=2, name="ostage")
                nc.vector.tensor_add(ostage, pp[:, :C], part[t])
                nc.sync.dma_start(out=out[t * 128:(t + 1) * 128, :], in_=ostage)

            # ---------------- schedule ----------------
            wq_tiles = {}

            def wq_fetch(m, eng=None):
                wq_tiles[m] = wqp.tile([128, NK, 128], f32r, tag="wq", name=f"wq{m}")
                (eng or nc.sync).dma_start(out=wq_tiles[m], in_=wq[m, :, :, :])

            # Head: DMA issue is ~0.65us per descriptor per engine queue, so
            # split between the sync and scalar queues (scalar is idle until
            # the first exp).  x streams per-kk on sync in QK(0)'s consumption
            # order; weights/constants go on scalar.
            for m in (0, 6, 1, 7):
                wq_fetch(m, nc.scalar)
            nc.scalar.dma_start(out=msk_t, in_=msk[:, :])
            nc.scalar.dma_start(out=ones_t, in_=onesc[:, :])
            nc.scalar.dma_start(out=sel_t, in_=sel[:, :])
            for kk in range(NK):
                nc.sync.dma_start(out=xT_t[kk][:, 0:512], in_=xTd[:, kk, 0:512])
                nc.sync.dma_start(out=xT_t[kk][:, 512:1024], in_=xTd[:, kk, 512:1024])
            nc.sync.dma_start(out=wvd, in_=wvs)
            nc.sync.dma_start(
                out=wpall.rearrange("p (i n) -> p i n", i=NK),
                in_=wp.rearrange("i p n -> p i n"),
            )

            # ---- pre-alpha: first four QK tiles + first four v tiles ----
            qk_start(0, wq_tiles.pop(0), range(NK))
            qk_finish(0)
            qk_start(6, wq_tiles.pop(6), range(NK))
            qk_finish(6)
            qk_start(1, wq_tiles.pop(1), range(NK))
            qk_finish(1)
            qk_start(7, wq_tiles.pop(7), range(NK))
            qk_finish(7)
            for t in range(4):
                v_tile(t)
                wq_fetch((2, 8, 3, 9)[t])

            # ---- alpha: query half 0 attention; QK for hp+2 as filler ----
            for hp in range(NK):
                S(hp, 0, 0)
                S(hp, 0, 1)
                if hp <= 3:
                    m = hp + 2
                    qk_start(m, wq_tiles[m], range(3))
                    qk_start(m, wq_tiles.pop(m), range(3, NK))
                    qk_finish(m)
                elif hp == 4:
                    v_tile(4)
                else:
                    v_tile(6)
                A(hp, 0, 0, 4)
                S(hp, 0, 2)
                if hp > 0:
                    FIN_norm(hp - 1, 0)
                A(hp, 0, 1, 4)
                S(hp, 0, 3)
                if hp <= 3:
                    m = NK + hp + 2
                    qk_start(m, wq_tiles[m], range(3))
                    qk_start(m, wq_tiles.pop(m), range(3, NK))
                    qk_finish(m)
                    if hp < 2:
                        wq_fetch(hp + 4)
                        wq_fetch(NK + hp + 4)
                elif hp == 4:
                    v_tile(5)
                A(hp, 0, 2, 4)
                A(hp, 0, 3, 4)
                FIN_stage(hp, 0)

            # ---- beta: query half 1 attention + projection filler ----
            beta_fill = {
                0: [lambda: v_tile(7), lambda: Pa(0), lambda: Pa(1)],
                1: [lambda: Pa(2), lambda: Pa(3), lambda: Pb(0, 3, fil)],
                2: [lambda: Pb(1, 3, fil), lambda: Pb(2, 3, fil), lambda: Pb(3, 3, fil)],
                3: [None, lambda: Pa(4), lambda: Pa(5)],
                4: [lambda: Pa(6), lambda: Pa(7), None],
                5: [None, lambda: P2a(4), lambda: P2a(5)],
            }
            for hp in range(NK):
                fills = list(beta_fill[hp])

                def fill():
                    if fills:
                        f = fills.pop(0)
                        if f is not None:
                            f()

                S(hp, 1, 0)
                S(hp, 1, 1)
                fill()
                A(hp, 1, 0, 8)
                S(hp, 1, 2)
                A(hp, 1, 1, 8)
                if hp == 0:
                    FIN_norm(5, 0)
                else:
                    FIN_norm(hp - 1, 1)
                S(hp, 1, 3)
                fill()
                A(hp, 1, 2, 8)
                S(hp, 1, 4)
                A(hp, 1, 3, 8)
                S(hp, 1, 5)
                fill()
                A(hp, 1, 4, 8)
                S(hp, 1, 6)
                A(hp, 1, 5, 8)
                S(hp, 1, 7)
                A(hp, 1, 6, 8)
                A(hp, 1, 7, 8)
                FIN_stage(hp, 1, sums_first=(hp == 5))

            FIN_norm(5, 1)
            Pb(4, 5, spp)
            Pb(5, 5, spp)
            Pb(6, 3, spp)
            Pb(7, 3, spp)

    nc.compile()
    return nc


_nc = None


def _get_nc():
    global _nc
    if _nc is None:
        _nc = build()
    return _nc


def _host_prep(w_attn, w_proj):
    wq = np.ascontiguousarray(
        w_attn[:, :2 * C].reshape(NK, 128, 2 * NK, 128).transpose(2, 1, 0, 3)
    )
    wv_aug = np.zeros((C, H, DA), np.float32)
    wv_aug[:, :, :D] = w_attn[:, 2 * C:].reshape(C, H, D)
    wv = np.ascontiguousarray(wv_aug.reshape(NK, 128, VW))
    wp = np.ascontiguousarray(w_proj.reshape(NK, 128, C))
    tri = np.triu(np.ones((128, 128), np.float32))
    msk = np.concatenate([tri, tri], axis=1).astype(ml_dtypes.bfloat16)
    onesc = np.ones((128, H), ml_dtypes.bfloat16)
    sel = np.zeros((2, C), np.float32)
    for p in range(C):
        sel[(p % 128) // 64, p] = 1.0
    return wq, wv, wp, msk, onesc, sel


def kernel(x, w_attn, w_proj):
    x = np.asarray(x, dtype=np.float32)
    w_attn = np.asarray(w_attn, dtype=np.float32)
    w_proj = np.asarray(w_proj, dtype=np.float32)
    wq, wv, wp, msk, onesc, sel = _host_prep(w_attn, w_proj)
    in_maps = [
        {
            "xT": np.ascontiguousarray(x[b].T),
            "wq": wq,
            "wv": wv,
            "wp": wp,
            "msk": msk,
            "onesc": onesc,
            "sel": sel,
        }
        for b in range(B)
    ]
    last_err = None
    for _attempt in range(3):
        try:
            res = run_bass_kernel_spmd(_get_nc(), in_maps, list(range(B)))
            return np.stack([res.results[b]["out"] for b in range(B)], axis=0)
        except Exception as e:  # transient device wedge: retry
            last_err = e
    raise last_err
